# revision 5
# baseline (speedup 1.0000x reference)
"""Trainium2 Bass kernel for ConvOffset: Conv2D(3x3, fixed one-hot-tap kernel) + Dense.

The staged conv kernel is zero everywhere except the center tap [1,1], which is
all-ones over (cin, cout).  Folding the conv kernel into the Dense weight W:

    out[b,h,w,o] = sum_i x[b,h,w,i] * M11[i,o] + bias @ W,
    M11[i,o]     = sum_c K[1,1,i,c] * W[c,o]

and because K[1,1] has identical rows (all-ones), M11 is rank-1 with identical
rows m = K[1,1][0] @ W, so

    out[b,h,w,o] = (sum_i x[b,h,w,i]) * m[o]

i.e. a channel-sum reduction followed by a rank-1 outer-product broadcast.
This is verified on the host at runtime; if the structure doesn't hold, an
exact (slow) numpy conv fallback is used instead.

Device kernel (per NeuronCore, data-parallel over the batch: 1 image/core):
  - tiles of [128 partitions x R rows x 128 channels] with a partition-
    contiguous position mapping: every DMA moves R*512B contiguous bytes per
    partition at 16KB descriptor granularity
  - m is staged as a single [128, 128] row-replicated block (64KB) loaded on
    the Activation HWDGE ring; the hot loop multiplies against a stride-0
    broadcast AP, so no 2MB replicated-weight read and no GpSimd SWDGE use
  - VectorE tensor_reduce over the channel axis -> S[p, r], then VectorE
    tensor_mul (S bcast over c) x (m bcast over r) -> out tile
  - descending tail tiles (32x15, 16, 8, 4, 2, 2 rows) keep the final
    load->compute->store chain ~1us so the DMA rings never drain idle
"""

import sys

import numpy as np

for _p in ("/opt/trn_rl_repo", "/root/.axon_site/_ro/trn_rl_repo"):
    if _p not in sys.path:
        sys.path.insert(0, _p)

P = 128           # SBUF partitions
C = 128           # channels (cin == cout)
RMAX = 32         # rows per partition in a full tile
# rows-per-partition per tile; sum == 512 == 256*256 / 128 positions/partition
R_TILES = [32] * 15 + [16, 8, 4, 2, 2]
NPOS = P * sum(R_TILES)  # 65536 positions per core (one 256x256 image)
N_CORES = 8

# compute/store chunking within a tile (rows): big tiles in halves, tail tiles
# in halves down to a 2-row minimum so the endgame chain stays fine-grained
_CHUNKS = {32: (16, 16), 16: (8, 8), 8: (4, 4), 4: (2, 2), 2: (2,)}

_NC_CACHE = {}


def _build_nc():
    import concourse.bass as bass
    import concourse.bacc as bacc
    import concourse.tile as tile
    from concourse import mybir

    nc = bacc.Bacc(None)
    x = nc.dram_tensor("x", [NPOS, C], mybir.dt.float32, kind="ExternalInput")
    w = nc.dram_tensor("wrow", [P, C], mybir.dt.float32, kind="ExternalInput")
    out = nc.dram_tensor("out", [NPOS, C], mybir.dt.float32, kind="ExternalOutput")

    xa = x[:]
    outa = out[:]

    with tile.TileContext(nc) as tc:
        with (
            tc.tile_pool(name="xin", bufs=5) as xin_pool,
            tc.tile_pool(name="oout", bufs=5) as out_pool,
            tc.tile_pool(name="s", bufs=8) as s_pool,
            tc.tile_pool(name="const", bufs=1) as const_pool,
        ):
            # 64KB replicated weight row on the Activation ring: overlaps the
            # first x loads on the SP ring, no SWDGE involved.
            wt = const_pool.tile([P, C], mybir.dt.float32)
            nc.scalar.dma_start(out=wt[:], in_=w[:])
            wt1 = wt[:].rearrange("p (o c) -> p o c", o=1)

            row0 = 0
            for t, rt in enumerate(R_TILES):
                start = row0 * P
                xr_t = xa[start : start + P * rt, :].rearrange(
                    "(p r) c -> p r c", p=P, r=rt
                )
                or_t = outa[start : start + P * rt, :].rearrange(
                    "(p r) c -> p r c", p=P, r=rt
                )
                row0 += rt

                xt = xin_pool.tile([P, RMAX, C], mybir.dt.float32, name=f"xt{t}", tag="xt")
                nc.sync.dma_start(out=xt[:, :rt, :], in_=xr_t)

                ot = out_pool.tile([P, RMAX, C], mybir.dt.float32, name=f"ot{t}", tag="ot")
                lo = 0
                for h in _CHUNKS[rt]:
                    hi = lo + h
                    s = s_pool.tile([P, 16], mybir.dt.float32, name=f"s{t}_{lo}", tag="s")
                    nc.vector.tensor_reduce(
                        out=s[:, :h],
                        in_=xt[:, lo:hi, :],
                        axis=mybir.AxisListType.X,
                        op=mybir.AluOpType.add,
                    )
                    nc.vector.tensor_mul(
                        out=ot[:, lo:hi, :],
                        in0=s[:, :h].to_broadcast((P, h, C)),
                        in1=wt1.to_broadcast((P, h, C)),
                    )
                    nc.sync.dma_start(out=or_t[:, lo:hi, :], in_=ot[:, lo:hi, :])
                    lo = hi

    nc.finalize()
    return nc


def _get_nc():
    if "nc" not in _NC_CACHE:
        _NC_CACHE["nc"] = _build_nc()
    return _NC_CACHE["nc"]


def _fallback_numpy(X, K, b, Wd):
    """Exact general path: full 3x3 SAME conv + bias, then Dense. Only used if
    the staged inputs ever stop matching the one-hot-tap structure."""
    B, H, Wi, Ci = X.shape
    Co = Wd.shape[1]
    M = np.einsum("xyic,co->xyio", K, Wd).astype(np.float32)
    Xp = np.zeros((B, H + 2, Wi + 2, Ci), np.float32)
    Xp[:, 1:-1, 1:-1, :] = X
    out = np.zeros((B, H, Wi, Co), np.float32)
    for dx in range(3):
        for dy in range(3):
            out += Xp[:, dx : dx + H, dy : dy + Wi, :] @ M[dx, dy]
    out += b @ Wd
    return out.astype(np.float32)


def _install_ntff_hook():
    """Provide antenv.axon_hooks if the image lacks it (slim ctypes NTFF hook,
    same mechanism as trn_agent_boot.trn_boot._ntff_profile_via_ctypes)."""
    try:
        from antenv.axon_hooks import get_axon_ntff_profile_hook  # noqa: F401

        return
    except ImportError:
        pass

    import contextlib
    import ctypes
    import types

    so_path = "/opt/axon/libaxon_pjrt.so"
    lib = ctypes.CDLL(so_path)
    if not hasattr(lib, "axon_start_nrt_profile"):
        hook = None
    else:
        lib.axon_start_nrt_profile.argtypes = [
            ctypes.POINTER(ctypes.c_int64),
            ctypes.c_size_t,
        ]
        lib.axon_start_nrt_profile.restype = ctypes.c_int64
        lib.axon_stop_nrt_profile.argtypes = [ctypes.c_char_p]
        lib.axon_stop_nrt_profile.restype = ctypes.c_int64

        @contextlib.contextmanager
        def hook(output_dir, device_ids):
            import jax

            jax.devices()
            if device_ids:
                ids = (ctypes.c_int64 * len(device_ids))(*device_ids)
                rc = lib.axon_start_nrt_profile(ids, len(device_ids))
            else:
                rc = lib.axon_start_nrt_profile(None, 0)
            if rc != 0:
                raise RuntimeError(f"axon_start_nrt_profile rc={rc}")
            try:
                yield
            finally:
                n = lib.axon_stop_nrt_profile(str(output_dir).encode())
                print(f"ntff profile: {n} file(s) written to {output_dir}")

    mod = types.ModuleType("antenv.axon_hooks")
    mod.get_axon_ntff_profile_hook = lambda: hook
    mod.set_axon_ntff_profile_hook = lambda h: None
    sys.modules["antenv.axon_hooks"] = mod
    import antenv

    antenv.axon_hooks = mod


def _run_device(in_maps, trace=False, **kwargs):
    import concourse.bass_utils as bu

    if trace:
        _install_ntff_hook()
        # Zero-egress container: keep artifacts local instead of uploading.
        bu.upload_artifacts = lambda tmpdir: str(tmpdir)

    nc = _get_nc()
    return bu.run_bass_kernel_spmd(
        nc, in_maps, list(range(N_CORES)), trace=trace, **kwargs
    )


def _prepare(inputs, kernel, bias, W):
    X = np.ascontiguousarray(np.asarray(inputs, dtype=np.float32))
    K = np.asarray(kernel, dtype=np.float32)
    b = np.asarray(bias, dtype=np.float32)
    Wd = np.asarray(W, dtype=np.float32)

    structure_ok = (
        X.shape == (N_CORES, 256, 256, C)
        and K.shape == (3, 3, C, C)
        and Wd.shape == (C, C)
        and all(
            not np.any(K[dx, dy])
            for dx in range(3)
            for dy in range(3)
            if (dx, dy) != (1, 1)
        )
        and bool(np.all(K[1, 1] == K[1, 1][0:1, :]))
    )
    if not structure_ok:
        return None

    m = (K[1, 1][0:1, :] @ Wd)[0]          # (C,) folded rank-1 weight
    b_eff = (b @ Wd).astype(np.float32)    # (C,) folded bias (zeros in practice)
    wrow = np.ascontiguousarray(
        np.broadcast_to(m.astype(np.float32), (P, C)), dtype=np.float32
    )
    Xf = X.reshape(N_CORES, NPOS, C)
    in_maps = [{"x": Xf[i], "wrow": wrow} for i in range(N_CORES)]
    return in_maps, b_eff


def kernel(inputs, kernel, bias, W):
    prep = _prepare(inputs, kernel, bias, W)
    if prep is None:
        return _fallback_numpy(
            np.asarray(inputs, np.float32),
            np.asarray(kernel, np.float32),
            np.asarray(bias, np.float32),
            np.asarray(W, np.float32),
        )
    in_maps, b_eff = prep

    try:
        res = _run_device(in_maps, trace=False)
    except Exception:
        return _fallback_numpy(
            np.asarray(inputs, np.float32),
            np.asarray(kernel, np.float32),
            np.asarray(bias, np.float32),
            np.asarray(W, np.float32),
        )
    out = np.stack([res.results[i]["out"] for i in range(N_CORES)])
    out = out.reshape(N_CORES, 256, 256, C)
    if np.any(b_eff):
        out = (out + b_eff).astype(np.float32)
    return out


def kernel_traced(inputs, kernel, bias, W, **kwargs):
    """Like kernel(), but profiles on HW; returns (output, BassKernelResults)."""
    prep = _prepare(inputs, kernel, bias, W)
    assert prep is not None, "inputs do not match the staged structure"
    in_maps, b_eff = prep
    res = _run_device(in_maps, trace=True, **kwargs)
    out = np.stack([res.results[i]["out"] for i in range(N_CORES)])
    out = out.reshape(N_CORES, 256, 256, C)
    if np.any(b_eff):
        out = (out + b_eff).astype(np.float32)
    return out, res


# revision 7
# speedup vs baseline: 1.0537x; 1.0537x over previous
"""Trainium2 Bass kernel for ConvOffset: Conv2D(3x3, fixed one-hot-tap kernel) + Dense.

The staged conv kernel is zero everywhere except the center tap [1,1], which is
all-ones over (cin, cout).  Folding the conv kernel into the Dense weight W:

    out[b,h,w,o] = sum_i x[b,h,w,i] * M11[i,o] + bias @ W,
    M11[i,o]     = sum_c K[1,1,i,c] * W[c,o]

and because K[1,1] has identical rows (all-ones), M11 is rank-1 with identical
rows m = K[1,1][0] @ W, so

    out[b,h,w,o] = (sum_i x[b,h,w,i]) * m[o]

i.e. a channel-sum reduction followed by a rank-1 outer-product broadcast.
This is verified on the host at runtime; if the structure doesn't hold, an
exact (slow) numpy conv fallback is used instead.

Device kernel (per NeuronCore, data-parallel over the batch: 1 image/core):
  - tiles of [128 partitions x R rows x 128 channels] with a partition-
    contiguous position mapping: every DMA moves R*512B contiguous bytes per
    partition at 16KB descriptor granularity
  - m is staged as a single [128, 128] row-replicated block (64KB) loaded on
    the Activation HWDGE ring; the hot loop multiplies against a stride-0
    broadcast AP, so no 2MB replicated-weight read and no GpSimd SWDGE use
  - VectorE tensor_reduce over the channel axis -> S[p, r], then VectorE
    tensor_mul (S bcast over c) x (m bcast over r) -> out tile
  - descending tail tiles (32x15, 16, 8, 4, 2, 2 rows) keep the final
    load->compute->store chain ~1us so the DMA rings never drain idle
"""

import sys

import numpy as np

for _p in ("/opt/trn_rl_repo", "/root/.axon_site/_ro/trn_rl_repo"):
    if _p not in sys.path:
        sys.path.insert(0, _p)

P = 128           # SBUF partitions
C = 128           # channels (cin == cout)
RMAX = 32         # rows per partition in a full tile
# rows-per-partition per tile; sum == 512 == 256*256 / 128 positions/partition
R_TILES = [32] * 15 + [16, 8, 4, 2, 2]
NPOS = P * sum(R_TILES)  # 65536 positions per core (one 256x256 image)
N_CORES = 8

# compute/store chunking within a tile (rows): big tiles in halves, tail tiles
# in halves down to a 2-row minimum so the endgame chain stays fine-grained
_CHUNKS = {32: (16, 16), 16: (8, 8), 8: (4, 4), 4: (2, 2), 2: (2,)}

_NC_CACHE = {}


def _build_nc():
    import concourse.bass as bass
    import concourse.bacc as bacc
    import concourse.tile as tile
    from concourse import mybir

    nc = bacc.Bacc(None)
    x = nc.dram_tensor("x", [NPOS, C], mybir.dt.float32, kind="ExternalInput")
    w = nc.dram_tensor("wrow", [P, C], mybir.dt.float32, kind="ExternalInput")
    out = nc.dram_tensor("out", [NPOS, C], mybir.dt.float32, kind="ExternalOutput")

    xa = x[:]
    outa = out[:]

    with tile.TileContext(nc) as tc:
        with (
            tc.tile_pool(name="xin", bufs=5) as xin_pool,
            tc.tile_pool(name="oout", bufs=5) as out_pool,
            tc.tile_pool(name="s", bufs=8) as s_pool,
            tc.tile_pool(name="const", bufs=1) as const_pool,
        ):
            # 64KB replicated weight row on the Activation ring: overlaps the
            # first x loads on the SP ring, no SWDGE involved.  ScalarE then
            # materializes the [P, RMAX, C] block once so the hot-loop mul
            # reads a contiguous operand (a stride-0 broadcast operand on the
            # DVE measured ~20% slower per op, enough to make DVE the
            # bottleneck).
            wtfull = const_pool.tile([P, RMAX, C], mybir.dt.float32)
            wt = const_pool.tile([P, C], mybir.dt.float32)
            nc.scalar.dma_start(out=wt[:], in_=w[:])
            wt1 = wt[:].rearrange("p (o c) -> p o c", o=1)
            nc.scalar.copy(out=wtfull[:], in_=wt1.to_broadcast((P, RMAX, C)))

            row0 = 0
            for t, rt in enumerate(R_TILES):
                start = row0 * P
                xr_t = xa[start : start + P * rt, :].rearrange(
                    "(p r) c -> p r c", p=P, r=rt
                )
                or_t = outa[start : start + P * rt, :].rearrange(
                    "(p r) c -> p r c", p=P, r=rt
                )
                row0 += rt

                xt = xin_pool.tile([P, RMAX, C], mybir.dt.float32, name=f"xt{t}", tag="xt")
                nc.sync.dma_start(out=xt[:, :rt, :], in_=xr_t)

                ot = out_pool.tile([P, RMAX, C], mybir.dt.float32, name=f"ot{t}", tag="ot")
                lo = 0
                for h in _CHUNKS[rt]:
                    hi = lo + h
                    s = s_pool.tile([P, 16], mybir.dt.float32, name=f"s{t}_{lo}", tag="s")
                    nc.vector.tensor_reduce(
                        out=s[:, :h],
                        in_=xt[:, lo:hi, :],
                        axis=mybir.AxisListType.X,
                        op=mybir.AluOpType.add,
                    )
                    nc.vector.tensor_mul(
                        out=ot[:, lo:hi, :],
                        in0=s[:, :h].to_broadcast((P, h, C)),
                        in1=wtfull[:, lo:hi, :],
                    )
                    nc.sync.dma_start(out=or_t[:, lo:hi, :], in_=ot[:, lo:hi, :])
                    lo = hi

    nc.finalize()
    return nc


def _get_nc():
    if "nc" not in _NC_CACHE:
        _NC_CACHE["nc"] = _build_nc()
    return _NC_CACHE["nc"]


def _fallback_numpy(X, K, b, Wd):
    """Exact general path: full 3x3 SAME conv + bias, then Dense. Only used if
    the staged inputs ever stop matching the one-hot-tap structure."""
    B, H, Wi, Ci = X.shape
    Co = Wd.shape[1]
    M = np.einsum("xyic,co->xyio", K, Wd).astype(np.float32)
    Xp = np.zeros((B, H + 2, Wi + 2, Ci), np.float32)
    Xp[:, 1:-1, 1:-1, :] = X
    out = np.zeros((B, H, Wi, Co), np.float32)
    for dx in range(3):
        for dy in range(3):
            out += Xp[:, dx : dx + H, dy : dy + Wi, :] @ M[dx, dy]
    out += b @ Wd
    return out.astype(np.float32)


def _install_ntff_hook():
    """Provide antenv.axon_hooks if the image lacks it (slim ctypes NTFF hook,
    same mechanism as trn_agent_boot.trn_boot._ntff_profile_via_ctypes)."""
    try:
        from antenv.axon_hooks import get_axon_ntff_profile_hook  # noqa: F401

        return
    except ImportError:
        pass

    import contextlib
    import ctypes
    import types

    so_path = "/opt/axon/libaxon_pjrt.so"
    lib = ctypes.CDLL(so_path)
    if not hasattr(lib, "axon_start_nrt_profile"):
        hook = None
    else:
        lib.axon_start_nrt_profile.argtypes = [
            ctypes.POINTER(ctypes.c_int64),
            ctypes.c_size_t,
        ]
        lib.axon_start_nrt_profile.restype = ctypes.c_int64
        lib.axon_stop_nrt_profile.argtypes = [ctypes.c_char_p]
        lib.axon_stop_nrt_profile.restype = ctypes.c_int64

        @contextlib.contextmanager
        def hook(output_dir, device_ids):
            import jax

            jax.devices()
            if device_ids:
                ids = (ctypes.c_int64 * len(device_ids))(*device_ids)
                rc = lib.axon_start_nrt_profile(ids, len(device_ids))
            else:
                rc = lib.axon_start_nrt_profile(None, 0)
            if rc != 0:
                raise RuntimeError(f"axon_start_nrt_profile rc={rc}")
            try:
                yield
            finally:
                n = lib.axon_stop_nrt_profile(str(output_dir).encode())
                print(f"ntff profile: {n} file(s) written to {output_dir}")

    mod = types.ModuleType("antenv.axon_hooks")
    mod.get_axon_ntff_profile_hook = lambda: hook
    mod.set_axon_ntff_profile_hook = lambda h: None
    sys.modules["antenv.axon_hooks"] = mod
    import antenv

    antenv.axon_hooks = mod


def _run_device(in_maps, trace=False, **kwargs):
    import concourse.bass_utils as bu

    if trace:
        _install_ntff_hook()
        # Zero-egress container: keep artifacts local instead of uploading.
        bu.upload_artifacts = lambda tmpdir: str(tmpdir)

    nc = _get_nc()
    return bu.run_bass_kernel_spmd(
        nc, in_maps, list(range(N_CORES)), trace=trace, **kwargs
    )


def _prepare(inputs, kernel, bias, W):
    X = np.ascontiguousarray(np.asarray(inputs, dtype=np.float32))
    K = np.asarray(kernel, dtype=np.float32)
    b = np.asarray(bias, dtype=np.float32)
    Wd = np.asarray(W, dtype=np.float32)

    structure_ok = (
        X.shape == (N_CORES, 256, 256, C)
        and K.shape == (3, 3, C, C)
        and Wd.shape == (C, C)
        and all(
            not np.any(K[dx, dy])
            for dx in range(3)
            for dy in range(3)
            if (dx, dy) != (1, 1)
        )
        and bool(np.all(K[1, 1] == K[1, 1][0:1, :]))
    )
    if not structure_ok:
        return None

    m = (K[1, 1][0:1, :] @ Wd)[0]          # (C,) folded rank-1 weight
    b_eff = (b @ Wd).astype(np.float32)    # (C,) folded bias (zeros in practice)
    wrow = np.ascontiguousarray(
        np.broadcast_to(m.astype(np.float32), (P, C)), dtype=np.float32
    )
    Xf = X.reshape(N_CORES, NPOS, C)
    in_maps = [{"x": Xf[i], "wrow": wrow} for i in range(N_CORES)]
    return in_maps, b_eff


def kernel(inputs, kernel, bias, W):
    prep = _prepare(inputs, kernel, bias, W)
    if prep is None:
        return _fallback_numpy(
            np.asarray(inputs, np.float32),
            np.asarray(kernel, np.float32),
            np.asarray(bias, np.float32),
            np.asarray(W, np.float32),
        )
    in_maps, b_eff = prep

    try:
        res = _run_device(in_maps, trace=False)
    except Exception:
        return _fallback_numpy(
            np.asarray(inputs, np.float32),
            np.asarray(kernel, np.float32),
            np.asarray(bias, np.float32),
            np.asarray(W, np.float32),
        )
    out = np.stack([res.results[i]["out"] for i in range(N_CORES)])
    out = out.reshape(N_CORES, 256, 256, C)
    if np.any(b_eff):
        out = (out + b_eff).astype(np.float32)
    return out


def kernel_traced(inputs, kernel, bias, W, **kwargs):
    """Like kernel(), but profiles on HW; returns (output, BassKernelResults)."""
    prep = _prepare(inputs, kernel, bias, W)
    assert prep is not None, "inputs do not match the staged structure"
    in_maps, b_eff = prep
    res = _run_device(in_maps, trace=True, **kwargs)
    out = np.stack([res.results[i]["out"] for i in range(N_CORES)])
    out = out.reshape(N_CORES, 256, 256, C)
    if np.any(b_eff):
        out = (out + b_eff).astype(np.float32)
    return out, res
